# revision 1
# baseline (speedup 1.0000x reference)
"""Trainium2 Bass kernel for the CrossAttention (linear-attention style) module.

Math (per batch b, head h):
    K = A @ Wk^T, V = A @ Wv^T  (A = input stream [N, C])
    ctx = softmax(scale * K^T V, axis=rows)          # [32, 32]
    out = Q @ ctx                                    # Q = A head-sliced

Key identity used here: K^T V = Wk (A^T A) Wv^T, so we only need the Gram
matrix G = A^T A (one [256,256] per batch/stream) from the big inputs; the
rest is tiny.  out = Q @ blockdiag(ctx_heads).

Sharding: 8 cores = 4 batches x 2 head-halves. Each core reads its batch's
rgb + x (channel-permuted so its own q-channels are always cols 0:128),
computes G (shared work duplicated across the pair), per-head ctx + softmax
on-chip, and writes its [16384, 128] output slice per stream.

The streamed data is fed as float32r (fp32 rounded to 11-bit mantissa, low
12 bits zero — rounding done host-side) so the PE runs matmuls at full
bf16-class rate instead of 1/4-rate fp32.

Passes per core:
  1) stream x[b]:  Gram_x accumulation + PE-transpose q_x tiles (retained)
  2) ctx_x; stream rgb[b]: Gram_r + transpose q_r + fused out_rgb matmuls
  3) ctx_rgb; out_x from retained q_x^T  (no HBM reads)
"""

import sys

if "/opt/trn_rl_repo" not in sys.path:
    sys.path.insert(0, "/opt/trn_rl_repo")

import numpy as np

import concourse.bass as bass
import concourse.mybir as mybir
import concourse.tile as tile
from concourse import bacc
from concourse.bass import ds, ts
from concourse.bass_utils import run_bass_kernel_spmd

P = 128
C = 256
HD = 32
NH_HALF = 4
SCALE = HD ** -0.5
F32 = mybir.dt.float32
F32R = mybir.dt.float32r

B_FULL = 4
N_FULL = 16384
H_FULL = 8


def round_to_f32r(a):
    """Round fp32 array to the FP32R grid (11-bit mantissa, RNE, low 12 bits 0)."""
    u = np.ascontiguousarray(a, dtype=np.float32).view(np.uint32)
    lsb = (u >> 12) & 1
    u = u + 0x7FF + lsb
    u &= 0xFFFFF000
    return u.view(np.float32)


def build_module(n_tok=N_FULL, t_chunk=2048, use_f32r=True, num_devices=8):
    DTS = F32R if use_f32r else F32  # dtype of the streamed data path
    nc = bacc.Bacc(
        "TRN2",
        target_bir_lowering=False,
        debug=False,
        enable_asserts=False,
        num_devices=num_devices,
    )
    a_x = nc.dram_tensor("a_x", [n_tok, C], DTS, kind="ExternalInput").ap()
    a_r = nc.dram_tensor("a_r", [n_tok, C], DTS, kind="ExternalInput").ap()
    wT_x = nc.dram_tensor("wT_x", [C, C], F32, kind="ExternalInput").ap()
    wT_r = nc.dram_tensor("wT_r", [C, C], F32, kind="ExternalInput").ap()
    ident = nc.dram_tensor("ident", [P, P], DTS, kind="ExternalInput").ap()
    o_r = nc.dram_tensor("o_r", [n_tok, P], F32, kind="ExternalOutput").ap()
    o_x = nc.dram_tensor("o_x", [n_tok, P], F32, kind="ExternalOutput").ap()

    with tile.TileContext(nc) as tc:
        _build_kernel(
            tc, a_x, a_r, wT_x, wT_r, ident, o_r, o_x, n_tok, t_chunk, DTS
        )
    nc.compile()
    return nc


def _build_kernel(tc, a_x, a_r, wT_x, wT_r, ident_d, o_r, o_x, n_tok, t_chunk, DTS):
    nc = tc.nc
    tiles_per_chunk = t_chunk // P
    n_chunks = n_tok // t_chunk
    n_tiles = n_tok // P

    ax_t = a_x.rearrange("(o p) c -> p o c", p=P)  # [128, n_tiles, 256]
    ar_t = a_r.rearrange("(o p) c -> p o c", p=P)
    or_t = o_r.rearrange("(o p) c -> p o c", p=P)  # [128, n_tiles, 128]
    ox_t = o_x.rearrange("(o p) c -> p o c", p=P)

    with (
        tc.tile_pool(name="persist", bufs=1) as persist,
        tc.tile_pool(name="chunks", bufs=3) as chunks,
        tc.tile_pool(name="qtr", bufs=3) as qtr_pool,
        tc.tile_pool(name="outs", bufs=3) as outs,
        tc.tile_pool(name="small", bufs=2) as small,
        tc.tile_pool(name="psum_g", bufs=1, space="PSUM") as psum_g,
        tc.tile_pool(name="psum_t", bufs=2, space="PSUM") as psum_t,
        tc.tile_pool(name="psum_o", bufs=2, space="PSUM") as psum_o,
        tc.tile_pool(name="psum_s", bufs=1, space="PSUM") as psum_s,
    ):
        # ---- persistent state ----
        qTx = persist.tile([P, n_tiles, P], DTS, tag="qTx")  # retained q_x^T
        g_x = persist.tile([P, 2, C], F32, tag="g_x")  # Gram accumulators
        g_r = persist.tile([P, 2, C], F32, tag="g_r")
        w_x = persist.tile([P, 2, C], F32, tag="w_x")  # [Wk_h^T | Wv_h^T]
        w_r = persist.tile([P, 2, C], F32, tag="w_r")
        ident = persist.tile([P, P], DTS, tag="ident")
        # rhs blockdiag ctx tiles, padded to 256 cols (zeros) for f32r speed
        rhs_x = persist.tile([P, C], DTS, tag="rhs_x")
        rhs_r = persist.tile([P, C], DTS, tag="rhs_r")

        nc.sync.dma_start(w_x[:], wT_x.rearrange("(o p) j -> p o j", p=P))
        nc.sync.dma_start(w_r[:], wT_r.rearrange("(o p) j -> p o j", p=P))
        nc.sync.dma_start(ident[:], ident_d)
        nc.vector.memset(g_x[:], 0.0)
        nc.vector.memset(g_r[:], 0.0)
        # DVE memset rejects the f32r dtype at ISA level; zero via uint32 view
        nc.vector.memset(rhs_x[:].bitcast(mybir.dt.uint32), 0)
        nc.vector.memset(rhs_r[:].bitcast(mybir.dt.uint32), 0)

        def stream_pass(a_t, g_sb, rhs, out_t, retain_qT):
            """One pass over an input stream: Gram accumulate, q^T transpose,
            and (if rhs is not None) fused out matmuls + output DMA."""
            for ch in range(n_chunks):
                in_sb = chunks.tile([P, tiles_per_chunk, C], DTS, tag="chunk")
                nc.sync.dma_start(in_sb[:], a_t[:, ts(ch, tiles_per_chunk), :])
                out_sb = None
                if rhs is not None:
                    out_sb = outs.tile([P, tiles_per_chunk, P], F32, tag="o_stage")
                # Gram: accumulate chunk into PSUM, then add to SBUF accumulator
                pg = [
                    psum_g.tile([P, C], F32, tag=f"g{i}", name=f"pg{i}")
                    for i in range(2)
                ]
                for t in range(tiles_per_chunk):
                    tile_ap = in_sb[:, t, :]  # [128 tok, 256 ch]
                    for i in range(2):
                        nc.tensor.matmul(
                            pg[i][:],
                            tile_ap[:, ts(i, P)],
                            tile_ap,
                            start=(t == 0),
                            stop=(t == tiles_per_chunk - 1),
                        )
                for i in range(2):
                    nc.vector.tensor_add(g_sb[:, i, :], g_sb[:, i, :], pg[i][:])
                # q^T transposes (+ fused out matmuls on pass 2)
                for t in range(tiles_per_chunk):
                    ti = ch * tiles_per_chunk + t
                    tp = psum_t.tile([P, P], DTS, tag="tp")
                    nc.tensor.transpose(tp[:], in_sb[:, t, :P], ident[:])
                    if retain_qT:
                        qT_tile = qTx[:, ti, :]
                    else:
                        qT_tile = qtr_pool.tile([P, P], DTS, tag="qTr")
                    nc.vector.tensor_copy(qT_tile, tp[:])
                    if rhs is not None:
                        po = psum_o.tile([P, C], F32, tag="out")
                        nc.tensor.matmul(
                            po[:], qT_tile, rhs[:], start=True, stop=True
                        )
                        nc.vector.tensor_copy(out_sb[:, t, :], po[:, :P])
                if rhs is not None:
                    nc.sync.dma_start(
                        out_t[:, ts(ch, tiles_per_chunk), :], out_sb[:]
                    )

        def compute_ctx(g_sb, w_sb, rhs):
            """rhs[32h+d, 32h+e] = softmax_d(scale * (Wk_h G Wv_h^T)) per head."""
            for h in range(NH_HALF):
                # tmpT[e', d] = sum_c G[c, e'] Wk_h^T[c, d]   (G symmetric)
                tmpT_ps = psum_s.tile([P, 2, HD], F32, tag="tmpT")
                for blk in range(2):
                    for ci in range(2):
                        nc.tensor.matmul(
                            tmpT_ps[:, blk, :],
                            g_sb[:, ci, ts(blk, P)],
                            w_sb[:, ci, ts(h, HD)],
                            start=(ci == 0),
                            stop=(ci == 1),
                        )
                tmpT_sb = small.tile([P, 2, HD], F32, tag="tmpT_sb")
                nc.vector.tensor_copy(tmpT_sb[:], tmpT_ps[:])
                # ctxT[e, d] = sum_e' Wv_h^T[e', e] tmpT[e', d]
                ctxT_ps = psum_s.tile([HD, HD], F32, tag="ctxT")
                for ci in range(2):
                    nc.tensor.matmul(
                        ctxT_ps[:],
                        w_sb[:, ci, ds(C // 2 + h * HD, HD)],
                        tmpT_sb[:, ci, :],
                        start=(ci == 0),
                        stop=(ci == 1),
                    )
                # softmax over d (free dim) with scale folded into exp
                mx = small.tile([HD, 1], F32, tag="mx")
                nc.vector.tensor_reduce(
                    mx[:], ctxT_ps[:], axis=mybir.AxisListType.X, op=mybir.AluOpType.max
                )
                nmx = small.tile([HD, 1], F32, tag="nmx")
                nc.vector.tensor_scalar_mul(nmx[:], mx[:], -SCALE)
                sm = small.tile([HD, HD], F32, tag="sm")
                ssum = small.tile([HD, 1], F32, tag="ssum")
                nc.scalar.activation(
                    sm[:],
                    ctxT_ps[:],
                    mybir.ActivationFunctionType.Exp,
                    bias=nmx[:],
                    scale=SCALE,
                    accum_out=ssum[:],
                )
                rs = small.tile([HD, 1], F32, tag="rs")
                nc.vector.reciprocal(rs[:], ssum[:])
                smn = small.tile([HD, HD], F32, tag="smn")
                nc.vector.tensor_scalar_mul(smn[:], sm[:], rs[:])
                # transpose [e, d] -> [d, e], then cast into blockdiag rhs slot
                nat = small.tile([HD, HD], F32, tag="nat")
                nc.vector.transpose(nat[:], smn[:])
                nc.vector.tensor_copy(rhs[ds(h * HD, HD), ds(h * HD, HD)], nat[:])

        # pass 1: x stream (Gram_x, retain q_x^T)
        stream_pass(ax_t, g_x, None, None, retain_qT=True)
        compute_ctx(g_x, w_x, rhs_x)
        # pass 2: rgb stream (Gram_r, out_rgb fused using ctx_x)
        stream_pass(ar_t, g_r, rhs_x, or_t, retain_qT=False)
        compute_ctx(g_r, w_r, rhs_r)
        # pass 3: out_x from retained q_x^T and ctx_rgb
        for ch in range(n_chunks):
            out_sb = outs.tile([P, tiles_per_chunk, P], F32, tag="o_stage")
            for t in range(tiles_per_chunk):
                ti = ch * tiles_per_chunk + t
                po = psum_o.tile([P, C], F32, tag="out")
                nc.tensor.matmul(
                    po[:], qTx[:, ti, :], rhs_r[:], start=True, stop=True
                )
                nc.vector.tensor_copy(out_sb[:, t, :], po[:, :P])
            nc.sync.dma_start(ox_t[:, ts(ch, tiles_per_chunk), :], out_sb[:])


# ---------------------------------------------------------------------------
# Host-side wrapper
# ---------------------------------------------------------------------------

_NC_CACHE = {}


def _get_module(**kw):
    key = tuple(sorted(kw.items()))
    if key not in _NC_CACHE:
        _NC_CACHE[key] = build_module(**kw)
    return _NC_CACHE[key]


def make_in_maps(rgb, x, Wkv_rgb, Wkv_x, n_cores=8, use_f32r=True):
    """Per-core input dicts. Core = (batch, head-half). Channels are permuted
    so each core's own q-channels sit in columns 0:128."""
    rnd = round_to_f32r if use_f32r else (lambda a: np.ascontiguousarray(a, np.float32))
    eye = np.eye(P, dtype=np.float32)
    in_maps = []
    for core in range(n_cores):
        b, hh = core // 2, core % 2
        perm = np.concatenate([np.arange(P * hh, C), np.arange(0, P * hh)])

        def wslice(W):
            Wk_h = W[P * hh : P * hh + P]  # [128, 256] rows (head-in-half, d)
            Wv_h = W[C + P * hh : C + P * hh + P]
            wT = np.concatenate([Wk_h.T, Wv_h.T], axis=1)  # [256 c, 256 j]
            return np.ascontiguousarray(wT[perm, :], dtype=np.float32)

        in_maps.append(
            {
                "a_x": rnd(x[b][:, perm]),
                "a_r": rnd(rgb[b][:, perm]),
                "wT_x": wslice(Wkv_x),
                "wT_r": wslice(Wkv_rgb),
                "ident": eye,
            }
        )
    return in_maps


def assemble(results):
    out_rgb = np.empty((B_FULL, N_FULL, C), dtype=np.float32)
    out_x = np.empty_like(out_rgb)
    for core, res in enumerate(results):
        b, hh = core // 2, core % 2
        out_rgb[b][:, P * hh : P * hh + P] = res["o_r"]
        out_x[b][:, P * hh : P * hh + P] = res["o_x"]
    return out_rgb, out_x


def kernel(rgb, x, Wkv_rgb, Wkv_x, num_heads):
    rgb = np.asarray(rgb, dtype=np.float32)
    x = np.asarray(x, dtype=np.float32)
    Wkv_rgb = np.asarray(Wkv_rgb, dtype=np.float32)
    Wkv_x = np.asarray(Wkv_x, dtype=np.float32)
    assert int(num_heads) == H_FULL
    assert rgb.shape == (B_FULL, N_FULL, C) and x.shape == (B_FULL, N_FULL, C)

    nc = _get_module()
    in_maps = make_in_maps(rgb, x, Wkv_rgb, Wkv_x)
    res = run_bass_kernel_spmd(nc, in_maps, core_ids=list(range(8)))
    return assemble(res.results)



# revision 4
# speedup vs baseline: 2.0255x; 2.0255x over previous
"""Trainium2 Bass kernel for the CrossAttention (linear-attention style) module.

Math (per batch b, head h, stream s in {rgb, x}):
    K = A_s @ Wk_s^T, V = A_s @ Wv_s^T            (A_s = stream input [N, C])
    ctx_s = softmax(scale * K^T V, axis=rows)     # [32, 32] per head
    out_s = A_s @ blockdiag(ctx_{s'})             # s' = the OTHER stream

Key identity: K^T V = Wk (A^T A) Wv^T, so the big inputs only feed the Gram
matrix G = A^T A (one [256,256] per (batch, stream)); the rest is tiny.

Sharding: 8 cores = 4 batches x 2 streams.  Core 2b+s owns stream s of
batch b: it streams A_s once (fp16, partition-major layout prepared on
host), computes G via PSUM-accumulated matmuls, derives its own ctx_s
on-chip, then swaps ctx with its pair core through a tiny AllReduce
(peer = sum - own, so the SPMD program stays core-independent).  While the
collective is in flight the PE transposes the retained input tiles
(plain matmuls against identity - NOT transpose-mode, which is slow).
Finally out^T = blockdiag(ctx_peer) block-stationary @ A^T in just
64 N=512 matmuls, written fp16 transposed; the host untransposes.

Hardware notes baked in:
 - Each Gram accumulation region gets its OWN PSUM bank: a start=True
   matmul clears has_written BANK-WIDE, so interleaving two accumulation
   groups in one bank corrupts the other region's in-flight tile
   (measured: split banks are exact to 1e-3 absolute; shared bank loses
   ~half a tile per group start).
 - PSUM->SBUF cast copies run ~1 elem/cycle/lane on both DVE and ACT,
   ~780 ns per [128,512]; they are the real phase gate, so they alternate
   between the two engines everywhere.

Everything on the streamed path is fp16: PE runs at full rate and DMA
traffic halves vs fp32 (8 MB in + 8 MB out per core).  Verified by a host
simulation of the exact pipeline: rel err ~5.6e-3 (gate is 2e-2).
"""

import sys

if "/opt/trn_rl_repo" not in sys.path:
    sys.path.insert(0, "/opt/trn_rl_repo")

import numpy as np

import concourse.bass as bass
import concourse.mybir as mybir
import concourse.tile as tile
from concourse import bacc
from concourse.bass import ds, ts
from concourse.bass_utils import run_bass_kernel_spmd

P = 128
C = 256
HD = 32
SCALE = HD ** -0.5
F16 = mybir.dt.float16
F32 = mybir.dt.float32

B_FULL = 4
N_FULL = 16384
H_FULL = 8

N_TILES = N_FULL // P          # 128
TPC = 16                       # tiles per chunk
N_CHUNKS = N_TILES // TPC      # 8
SPAN = 4                       # qT tiles per out-matmul -> N = 512

REPLICA_GROUPS = [[0, 1], [2, 3], [4, 5], [6, 7]]


def build_module(num_devices=8):
    nc = bacc.Bacc(
        "TRN2",
        target_bir_lowering=False,
        debug=False,
        enable_asserts=False,
        num_devices=num_devices,
    )
    a_pm = nc.dram_tensor("a_pm", [P, N_TILES, C], F16, kind="ExternalInput").ap()
    wkT = nc.dram_tensor("wkT", [P, 2, C], F16, kind="ExternalInput").ap()
    wvT = nc.dram_tensor("wvT", [P, 2, C], F16, kind="ExternalInput").ap()
    ident_d = nc.dram_tensor("ident", [P, P], F16, kind="ExternalInput").ap()
    oT = nc.dram_tensor("oT", [P, 2, N_FULL], F16, kind="ExternalOutput").ap()

    with tile.TileContext(nc) as tc:
        _build_kernel(tc, a_pm, wkT, wvT, ident_d, oT)
    nc.compile()
    return nc


def _build_kernel(tc, a_pm, wkT_d, wvT_d, ident_d, oT):
    nc = tc.nc

    with (
        tc.tile_pool(name="persist", bufs=1) as persist,
        tc.tile_pool(name="stage", bufs=3) as stage,
        tc.tile_pool(name="dram", bufs=1, space="DRAM") as dram,
        tc.tile_pool(name="psum_ga", bufs=1, space="PSUM") as psum_ga,
        tc.tile_pool(name="psum_gb", bufs=1, space="PSUM") as psum_gb,
        tc.tile_pool(name="psum_l", bufs=1, space="PSUM") as psum_l,
        tc.tile_pool(name="psum_t", bufs=2, space="PSUM") as psum_t,
        tc.tile_pool(name="psum_o", bufs=3, space="PSUM") as psum_o,
    ):
        # ---- persistent SBUF state ----
        in_sb = [
            persist.tile([P, TPC, C], F16, tag=f"in{ch}", name=f"in{ch}")
            for ch in range(N_CHUNKS)
        ]
        qT_sb = [
            persist.tile([P, 2, TPC, P], F16, tag=f"qT{ch}", name=f"qT{ch}")
            for ch in range(N_CHUNKS)
        ]
        w_k = persist.tile([P, 2, C], F16, tag="w_k")
        w_v = persist.tile([P, 2, C], F16, tag="w_v")
        ident = persist.tile([P, P], F16, tag="ident")
        g16 = persist.tile([P, 2, C], F16, tag="g16")
        T16 = persist.tile([P, 2, C], F16, tag="T16")
        lgc = persist.tile([P, 2, HD], F32, tag="lgc")    # compact logits^T
        cT_own = persist.tile([P, 2, HD], F32, tag="cT_own")
        cT_sum = persist.tile([P, 2, HD], F32, tag="cT_sum")
        cT_peer = persist.tile([P, 2, HD], F32, tag="cT_peer")
        ctx16 = persist.tile([P, 2, P], F16, tag="ctx16")  # blockdiag, fp16

        b_in = dram.tile([P, 2, HD], F32, tag="b_in", name="b_in")
        b_out = dram.tile([P, 2, HD], F32, tag="b_out", name="b_out")

        # ---- input stream first (chunk 0 split for an earlier Gram start) ----
        nc.sync.dma_start(in_sb[0][:, ts(0, TPC // 2), :], a_pm[:, ts(0, TPC // 2), :])
        nc.sync.dma_start(
            in_sb[0][:, ds(TPC // 2, TPC // 2), :], a_pm[:, ds(TPC // 2, TPC // 2), :]
        )
        for ch in range(1, N_CHUNKS):
            nc.sync.dma_start(in_sb[ch][:], a_pm[:, ts(ch, TPC), :])
        nc.sync.dma_start(w_k[:], wkT_d)
        nc.sync.dma_start(w_v[:], wvT_d)
        nc.sync.dma_start(ident[:], ident_d)
        nc.vector.memset(ctx16[:], 0.0)

        # ---- phase 1: Gram G = A^T A; one accumulation region PER BANK ----
        pga = psum_ga.tile([P, 2, C], F32, tag="ga", name="pga")  # G[0:128, :]
        pgb = psum_gb.tile([P, 2, C], F32, tag="gb", name="pgb")  # G[128:256, :]
        for ch in range(N_CHUNKS):
            for t in range(TPC):
                ti = ch * TPC + t
                tile_ap = in_sb[ch][:, t, :]
                nc.tensor.matmul(
                    pga[:, 0, :], tile_ap[:, ts(0, P)], tile_ap,
                    start=(ti == 0), stop=(ti == N_TILES - 1),
                )
                nc.tensor.matmul(
                    pgb[:, 0, :], tile_ap[:, ts(1, P)], tile_ap,
                    start=(ti == 0), stop=(ti == N_TILES - 1),
                )
        nc.vector.tensor_copy(g16[:, 0, :], pga[:, 0, :])
        nc.scalar.copy(g16[:, 1, :], pgb[:, 0, :])

        # ---- ctx_own: T = G @ Wk^T (G symmetric), logits = Wv @ T ----
        # pT blocks reuse the two Gram banks (same tag ring -> sequenced).
        pTA = psum_ga.tile([P, 2, C], F32, tag="ga", name="pTA")  # T[0:128, :]
        pTB = psum_gb.tile([P, 2, C], F32, tag="gb", name="pTB")  # T[128:256, :]
        for blkc, pT in ((0, pTA), (1, pTB)):
            for ci in range(2):
                nc.tensor.matmul(
                    pT[:, 0, :], g16[:, ci, ts(blkc, P)], w_k[:, ci, :],
                    start=(ci == 0), stop=(ci == 1),
                )
        nc.vector.tensor_copy(T16[:, 0, :], pTA[:, 0, :])
        nc.scalar.copy(T16[:, 1, :], pTB[:, 0, :])

        for g in range(2):
            pl = psum_l.tile([P, P], F32, tag="pl", name=f"pl{g}")
            for ci in range(2):
                nc.tensor.matmul(
                    pl[:], w_v[:, ci, ts(g, P)], T16[:, ci, ts(g, P)],
                    start=(ci == 0), stop=(ci == 1),
                )
            # extract the 4 diagonal head blocks -> compact [128, 32]
            for h in range(4):
                nc.vector.tensor_copy(
                    lgc[ds(h * HD, HD), g, :], pl[ds(h * HD, HD), ds(h * HD, HD)]
                )
            # batched softmax over d (free axis) for all 4 heads at once
            mx = stage.tile([P, 1], F32, tag="mx", name=f"mx{g}")
            nc.vector.tensor_reduce(
                mx[:], lgc[:, g, :], axis=mybir.AxisListType.X, op=mybir.AluOpType.max
            )
            nmx = stage.tile([P, 1], F32, tag="nmx", name=f"nmx{g}")
            nc.vector.tensor_scalar_mul(nmx[:], mx[:], -SCALE)
            sm = stage.tile([P, HD], F32, tag="sm", name=f"sm{g}")
            ssum = stage.tile([P, 1], F32, tag="ssum", name=f"ssum{g}")
            nc.scalar.activation(
                sm[:], lgc[:, g, :], mybir.ActivationFunctionType.Exp,
                bias=nmx[:], scale=SCALE, accum_out=ssum[:],
            )
            rs = stage.tile([P, 1], F32, tag="rs", name=f"rs{g}")
            nc.vector.reciprocal(rs[:], ssum[:])
            smn = stage.tile([P, HD], F32, tag="smn", name=f"smn{g}")
            nc.vector.tensor_scalar_mul(smn[:], sm[:], rs[:])
            # per-head 32x32 transpose: [32h+e, d] -> [32h+d, e]
            nc.vector.transpose(cT_own[:, g, :], smn[:])

        # ---- exchange ctx with the pair core (peer = pairsum - own) ----
        nc.gpsimd.dma_start(b_in[:], cT_own[:])
        nc.gpsimd.collective_compute(
            "AllReduce",
            mybir.AluOpType.add,
            replica_groups=REPLICA_GROUPS,
            ins=[b_in.opt()],
            outs=[b_out.opt()],
        )
        nc.gpsimd.dma_start(cT_sum[:], b_out[:])

        # ---- overlap the collective: PE transposes the retained input ----
        cp = 0
        for ch in range(N_CHUNKS):
            for tp in range(TPC // 2):
                pt = psum_t.tile([P, 2, 2, P], F32, tag="pt", name=f"pt{ch}_{tp}")
                for i in range(2):
                    for t2 in range(2):
                        nc.tensor.matmul(
                            pt[:, i, t2, :],
                            in_sb[ch][:, tp * 2 + t2, ts(i, P)],
                            ident[:], start=True, stop=True,
                        )
                dst = qT_sb[ch][:, :, ts(tp, 2), :]
                if cp % 2 == 0:
                    nc.vector.tensor_copy(dst, pt[:])
                else:
                    nc.scalar.copy(dst, pt[:])
                cp += 1

        nc.vector.tensor_sub(cT_peer[:], cT_sum[:], cT_own[:])
        # scatter-cast peer ctx into fp16 blockdiag form
        for g in range(2):
            for h in range(4):
                nc.vector.tensor_copy(
                    ctx16[ds(h * HD, HD), g, ds(h * HD, HD)],
                    cT_peer[ds(h * HD, HD), g, :],
                )

        # ---- out^T = ctx_blk (stationary) @ qT spans, fp16 staged, DMA out ----
        st = None
        for g in range(2):
            for q in range(N_TILES // SPAN):
                ch, sp = divmod(q, TPC // SPAN)
                po = psum_o.tile([P, SPAN * P], F32, tag="po", name=f"po{g}_{q}")
                nc.tensor.matmul(
                    po[:], ctx16[:, g, :], qT_sb[ch][:, g, ts(sp, SPAN), :],
                    start=True, stop=True,
                )
                if q % 2 == 0:
                    st = stage.tile(
                        [P, 2 * SPAN * P], F16, tag="st", name=f"st{g}_{q}"
                    )
                    nc.vector.tensor_copy(st[:, ts(0, SPAN * P)], po[:])
                else:
                    nc.scalar.copy(st[:, ts(1, SPAN * P)], po[:])
                    nc.sync.dma_start(oT[:, g, ts(q // 2, 2 * SPAN * P)], st[:])


# ---------------------------------------------------------------------------
# Host-side wrapper
# ---------------------------------------------------------------------------

_NC_CACHE = {}


def _get_module(**kw):
    key = tuple(sorted(kw.items()))
    if key not in _NC_CACHE:
        _NC_CACHE[key] = build_module(**kw)
    return _NC_CACHE[key]


def make_in_maps(rgb, x, Wkv_rgb, Wkv_x, n_cores=8):
    """Per-core inputs. Core 2b+s owns stream s (0=rgb, 1=x) of batch b."""
    eye = np.eye(P, dtype=np.float16)
    in_maps = []
    for core in range(n_cores):
        b, s = divmod(core, 2)
        A = (rgb if s == 0 else x)[b]
        W = Wkv_rgb if s == 0 else Wkv_x
        a16 = A.astype(np.float16)
        a_pm = np.ascontiguousarray(a16.reshape(N_TILES, P, C).transpose(1, 0, 2))
        WkT = W[:C].T.reshape(2, P, C).transpose(1, 0, 2)   # [p, ci, col]
        WvT = W[C:].T.reshape(2, P, C).transpose(1, 0, 2)
        in_maps.append(
            {
                "a_pm": a_pm,
                "wkT": np.ascontiguousarray(WkT.astype(np.float16)),
                "wvT": np.ascontiguousarray(WvT.astype(np.float16)),
                "ident": eye,
            }
        )
    return in_maps


def assemble(results):
    out_rgb = np.empty((B_FULL, N_FULL, C), dtype=np.float32)
    out_x = np.empty_like(out_rgb)
    for core, res in enumerate(results):
        b, s = divmod(core, 2)
        o = res["oT"].transpose(2, 1, 0).reshape(N_FULL, C).astype(np.float32)
        (out_rgb if s == 0 else out_x)[b] = o
    return out_rgb, out_x


def kernel(rgb, x, Wkv_rgb, Wkv_x, num_heads):
    rgb = np.asarray(rgb, dtype=np.float32)
    x = np.asarray(x, dtype=np.float32)
    Wkv_rgb = np.asarray(Wkv_rgb, dtype=np.float32)
    Wkv_x = np.asarray(Wkv_x, dtype=np.float32)
    assert int(num_heads) == H_FULL
    assert rgb.shape == (B_FULL, N_FULL, C) and x.shape == (B_FULL, N_FULL, C)

    nc = _get_module()
    in_maps = make_in_maps(rgb, x, Wkv_rgb, Wkv_x)
    res = run_bass_kernel_spmd(nc, in_maps, core_ids=list(range(8)))
    return assemble(res.results)
